# revision 25
# baseline (speedup 1.0000x reference)
"""Trainium2 Bass kernel for nn_DecoderBlock (B=1, S=2048, D=1024, H=16, DQ=64, DM=4096).

Strategy (8 NeuronCores, one chip):
  - Attention tensor-parallel over heads: core c owns heads {2c, 2c+1}.
    Per-head projections fused with QKV: weffT[din, e] = (H_x[h] @ W_x).T,
    computed on device transpose-free by contracting W chunks against H.T.
    Scores are computed TRANSPOSED ([keys, queries]) so softmax's reduction
    lands on the matmul contraction axis: a row of ones appended to v gives
    the softmax denominators for free.
  - Causality exploited: key-tiles strictly above the diagonal are skipped
    (never computed); diagonal tiles are masked in-place with a device-side
    affine_select on the GpSimd engine (no mask tensor, no mask DMA at all).
  - One AllToAll exchanges attention outputs (heads-sharded -> token-sharded).
  - Post-attention (W_O, LN1, MLP, LN2, residuals) data-parallel over tokens:
    core c owns tokens [c*256, (c+1)*256) in transposed [d, s] layout;
    LayerNorm statistics via ones-matmuls over the partition axis.
  - All weight/activation DMA payloads are bf16 (half the HBM traffic of
    f32/f32r); matmuls run bf16 with fp32 PSUM accumulation; the two
    residual adds and LN statistics chains stay fp32.
  - MLP weights (L1 full, L2 streamed) prefetch during attention so the DMA
    engines and PE overlap instead of serializing.
"""

import numpy as np
import ml_dtypes

BF16 = ml_dtypes.bfloat16

B, S_FULL, D, H, DQ, DM = 1, 2048, 1024, 16, 64, 4096
NC = 8          # cores
P = 128         # partitions
HPC = H // NC   # heads per core
EPC = HPC * DQ  # per-core attention width (128)
DK = D // P     # d-model chunks (8)
MK = DM // P    # mlp chunks (32)
EPS = 1e-5


def _body(tc, io, S, use_collective=True, stop_after=None):
    import concourse.bass as bass
    import concourse.mybir as mybir
    from concourse.masks import make_identity

    nc = tc.nc
    fp32 = mybir.dt.float32
    bf16 = mybir.dt.bfloat16
    Exp = mybir.ActivationFunctionType.Exp
    Relu = mybir.ActivationFunctionType.Relu
    Sqrt = mybir.ActivationFunctionType.Sqrt
    Copy = mybir.ActivationFunctionType.Copy
    sub_op = mybir.AluOpType.subtract
    mult_op = mybir.AluOpType.mult
    add_op = mybir.AluOpType.add
    ge_op = mybir.AluOpType.is_ge

    SL = S // NC                 # tokens per core (256)
    TT = S // P                  # key tiles (16)
    SPW = 512                    # attention query-strip width
    NSP = S // SPW               # 4
    TPS = SPW // P               # key tiles per strip width (4)
    QH = min(1024, S)            # qkv streaming half width
    NQH = S // QH                # 2
    QSP = QH // SPW              # strips per half (2)

    ts = bass.ts

    # ---------------- persistent SBUF ----------------
    import contextlib
    _ctx = contextlib.ExitStack()
    persist = _ctx.enter_context(tc.tile_pool(name="persist", bufs=1))

    def ptile(shape, dt, tag):
        return persist.tile(shape, dt, tag=tag, name=tag)

    drampool = _ctx.enter_context(tc.tile_pool(name="drampool", bufs=1, space="DRAM"))

    qt_sb = ptile([EPC, S], bf16, "qt_sb")
    kt_sb = ptile([EPC, S], bf16, "kt_sb")
    vaug_sb = ptile([P, HPC, TT, DQ + 1], bf16, "vaug_sb")
    ot_sb = ptile([EPC, S], bf16, "ot_sb")            # attn out (2 heads)
    weff_sb = ptile([P, 3, DK, EPC], bf16, "weff_sb")  # weffT[din_p, j, kk, e]
    onesb_sb = ptile([P, 1], bf16, "onesb_sb")
    gb_sb = ptile([P, 6, DK], fp32, "gb_sb")          # g1,b1,g2,b2,l2b,(pad)
    l1b_sb = ptile([P, MK], fp32, "l1b_sb")
    wot_sb = ptile([P, DK, D], bf16, "wot_sb")
    eres_sb = ptile([P, DK, SL], fp32, "eres_sb")
    eresb_sb = ptile([P, DK, SL], bf16, "eresb_sb")
    x1t_sb = ptile([P, DK, SL], fp32, "x1t_sb")
    x1b_sb = ptile([P, DK, SL], bf16, "x1b_sb")
    att_sb = ptile([P, DK, SL], bf16, "att_sb")
    mlp_sb = ptile([P, DK, SL], bf16, "mlp_sb")
    ots_sb = ptile([P, DK, SL], bf16, "ots_sb")       # o for my tokens

    tri_sb = ptile([P, TPS, SPW], bf16, "tri_sb")     # diagonal-tile causal masks

    # eT in strip-chunk DMAs on the ACT queue (sync queue carries weights);
    # qkv strip sp only needs chunk sp, so compute starts before eT finishes
    l1pool = _ctx.enter_context(tc.tile_pool(name="l1pool", bufs=2))
    l2pool = _ctx.enter_context(tc.tile_pool(name="l2pool", bufs=2))
    etctx = contextlib.ExitStack()
    etpool = etctx.enter_context(tc.tile_pool(name="etpool", bufs=1))
    et_sb = etpool.tile([P, DK, S], bf16, tag="et_sb", name="et_sb")

    nc.gpsimd.memset(onesb_sb[:], 1.0)
    nc.gpsimd.memset(vaug_sb[:, :, :, DQ:], 1.0)      # denominator rows
    nc.gpsimd.memset(tri_sb[:], 1.0)
    for j in range(TPS):   # keep (1.0) iff query >= key inside offset-j tile
        nc.gpsimd.affine_select(
            out=tri_sb[:, j, :], in_=tri_sb[:, j, :], compare_op=ge_op,
            fill=0.0, base=-P * j, channel_multiplier=-1, pattern=[[1, SPW]])

    # ---------------- weffT[j] = sum_dout W_chunk.T-contraction vs H.T --------
    with tc.tile_pool(name="htpool", bufs=1) as htpool, \
         tc.tile_pool(name="wpool", bufs=3) as wpool, \
         tc.tile_pool(name="wpsum", bufs=2, space="PSUM") as wpsum:
        ht = htpool.tile([P, 3, DK, EPC], bf16, tag="ht")
        for j, hn in enumerate(("htq", "htk", "htv")):
            nc.sync.dma_start(ht[:, j], io[hn].rearrange("(dh p) e -> p dh e", p=P))
        w_tiles = []
        for j, wn in enumerate(("wq", "wk", "wv")):
            w_all = wpool.tile([P, DK, D], bf16, tag="w_all", name="w_all")
            nc.sync.dma_start(w_all[:], io[wn].rearrange("(dh p) e -> p dh e", p=P))
            w_tiles.append(w_all)
            if j == 0:          # eT strip 0 right after wq, rest after wv
                nc.sync.dma_start(
                    et_sb[:, :, ts(0, SPW)],
                    io["eT"][:, ts(0, SPW)].rearrange("(kk p) s -> p kk s", p=P))
        for sp in range(1, S // SPW):
            nc.sync.dma_start(
                et_sb[:, :, ts(sp, SPW)],
                io["eT"][:, ts(sp, SPW)].rearrange("(kk p) s -> p kk s", p=P))
        for j in range(3):
            w_all = w_tiles[j]
            psT = wpsum.tile([P, DK, EPC], fp32, tag="psT")
            for kk in range(DK):
                for dh in range(DK):
                    nc.tensor.matmul(psT[:, kk, :], w_all[:, dh, ts(kk, P)],
                                     ht[:, j, dh, :],
                                     start=(dh == 0), stop=(dh == DK - 1))
            if j == 0:  # fold 1/sqrt(DQ) into q weights
                nc.scalar.activation(weff_sb[:, j], psT[:], Copy,
                                     scale=1.0 / float(np.sqrt(DQ)))
            else:
                nc.scalar.activation(weff_sb[:, j], psT[:], Copy)

    # -------- prefetch W_O + small tensors (DMA overlaps attention) ----------
    nc.sync.dma_start(wot_sb[:], io["wot"].rearrange("(g p) e -> p g e", p=P))
    for i, name in enumerate(("g1", "b1", "g2", "b2", "l2b")):
        nc.sync.dma_start(gb_sb[:, i, :], io[name].rearrange("(g p) -> p g", p=P))
    nc.sync.dma_start(l1b_sb[:], io["l1b"].rearrange("(g p) -> p g", p=P))
    nc.sync.dma_start(eres_sb[:], io["eresT"].rearrange("(g p) s -> p g s", p=P))
    nc.sync.dma_start(eresb_sb[:], io["eresb"].rearrange("(g p) s -> p g s", p=P))

    a2a_in = drampool.tile([NC * EPC, SL], bf16, tag="a2a_in", name="a2a_in")
    a2a_out = drampool.tile([NC * EPC, SL], bf16, tag="a2a_out", name="a2a_out")

    # ------- fused per-strip qkv projection + causal attention ---------------
    # Strip sp: project q/k for queries in the strip and v for its key tiles
    # (v computed directly in [keys, e] layout: eT-tile stationary, weffT
    # moving -- no transposes), then run attention for key tiles 0..4sp+3.
    # exp runs on ACT (the strip-phase bottleneck), all copies/masks on DVE,
    # o-matmuls lag score-matmuls by GD tiles so PE/ACT/DVE pipeline.
    GD = 2
    with tc.tile_pool(name="expool", bufs=6) as expool, \
         tc.tile_pool(name="scpsum", bufs=2, space="PSUM") as scpsum, \
         tc.tile_pool(name="opsum", bufs=1, space="PSUM") as opsum, \
         tc.tile_pool(name="qkpsum", bufs=1, space="PSUM") as qkpsum, \
         tc.tile_pool(name="pvpsum", bufs=1, space="PSUM") as pvpsum, \
         tc.tile_pool(name="nrmpool", bufs=2) as nrmpool:
        carry = None          # (sp, tmax, pso, exs) pending from previous strip

        def finish_strip(sp0, tmax0, pso0, exs0):
            qs0 = slice(sp0 * SPW, (sp0 + 1) * SPW)
            for t2 in (tmax0 - 2, tmax0 - 1):
                ex2 = exs0.pop(t2)
                for h in range(HPC):
                    nc.tensor.matmul(pso0[h][:], vaug_sb[:, h, t2, :],
                                     ex2[:, h, :],
                                     start=(t2 == 0), stop=(t2 == tmax0 - 1))
            for h in range(HPC):
                hs = slice(h * DQ, (h + 1) * DQ)
                rc = nrmpool.tile([1, SPW], fp32, tag="rc")
                nc.vector.reciprocal(rc[:], pso0[h][DQ:DQ + 1, :])
                rcb = nrmpool.tile([DQ, SPW], fp32, tag="rcb")
                nc.gpsimd.partition_broadcast(rcb[:], rc[:], channels=DQ)
                nc.vector.tensor_mul(ot_sb[hs, qs0], pso0[h][:DQ, :], rcb[:])
            # stage this strip's heads-block into the AllToAll source buffer
            nc.scalar.dma_start(
                a2a_in[HPC * sp0 * EPC:HPC * (sp0 + 1) * EPC, :].rearrange(
                    "(g e) s -> e g s", g=HPC),
                ot_sb[:, qs0].rearrange("e (g s) -> e g s", g=HPC))

        def emit_qkv(sp):
            qs = slice(sp * SPW, (sp + 1) * SPW)
            for j, dst in ((0, qt_sb), (1, kt_sb)):
                pj = qkpsum.tile([EPC, SPW], fp32, tag="pj", name="pj")
                for kk in range(DK):
                    nc.tensor.matmul(pj[:], weff_sb[:, j, kk, :],
                                     et_sb[:, kk, qs],
                                     start=(kk == 0), stop=(kk == DK - 1))
                nc.vector.tensor_copy(dst[:, qs], pj[:])
            for ti in range(TPS):
                t = TPS * sp + ti
                pv = pvpsum.tile([P, EPC], fp32, tag="pv", name="pv")
                for kk in range(DK):
                    nc.tensor.matmul(pv[:], et_sb[:, kk, ts(t, P)],
                                     weff_sb[:, 2, kk, :],
                                     start=(kk == 0), stop=(kk == DK - 1))
                nc.vector.tensor_copy(vaug_sb[:, :, t, :DQ],
                                      pv.rearrange("p (h e) -> p h e", h=HPC))

        for sp in range(NSP):
            qs = slice(sp * SPW, (sp + 1) * SPW)
            emit_qkv(sp)
            if carry is not None:
                finish_strip(*carry)
            tmax = TPS * (sp + 1)          # key tiles needed by this strip
            pso = [opsum.tile([DQ + 1, SPW], fp32, tag=f"pso{h}", name=f"pso{h}")
                   for h in range(HPC)]
            exs = {}
            for tt in range(tmax):
                if tt < tmax:
                    t = tt
                    ps = scpsum.tile([P, HPC, SPW], fp32, tag="scps")
                    for h in range(HPC):
                        hs = slice(h * DQ, (h + 1) * DQ)
                        nc.tensor.matmul(ps[:, h, :], kt_sb[hs, ts(t, P)],
                                         qt_sb[hs, qs], start=True, stop=True)
                    ex = expool.tile([P, HPC, SPW], bf16, tag="ex")
                    nc.scalar.activation(ex[:], ps[:], Exp)
                    if t >= TPS * sp:      # diagonal tile: causal mask in-place
                        for h in range(HPC):
                            nc.vector.tensor_mul(ex[:, h, :], ex[:, h, :],
                                                 tri_sb[:, t - TPS * sp, :])
                    exs[t] = ex
                if tt >= GD:
                    t2 = tt - GD
                    ex2 = exs.pop(t2)
                    for h in range(HPC):
                        nc.tensor.matmul(pso[h][:], vaug_sb[:, h, t2, :],
                                         ex2[:, h, :],
                                         start=(t2 == 0), stop=(t2 == tmax - 1))
            carry = (sp, tmax, pso, exs)
        finish_strip(*carry)
    etctx.close()

    if stop_after in ("qkv", "attn"):
        _ctx.close(); return

    # ---------------- AllToAll: heads-sharded -> token-sharded ----------------
    a2a_in = drampool.tile([NC * EPC, SL], bf16, tag="a2a_in", name="a2a_in")
    a2a_out = drampool.tile([NC * EPC, SL], bf16, tag="a2a_out", name="a2a_out")
    nc.scalar.dma_start(a2a_in.rearrange("(g e) s -> e g s", g=NC),
                        ot_sb.rearrange("e (g s) -> e g s", g=NC))
    if use_collective:
        nc.gpsimd.collective_compute(
            "AllToAll", mybir.AluOpType.bypass,
            replica_groups=[list(range(NC))],
            ins=[a2a_in.opt()], outs=[a2a_out.opt()])
    else:  # timing-only single-core variant (results wrong across cores)
        nc.scalar.dma_start(a2a_out[:], a2a_in[:])
    nc.scalar.dma_start(ots_sb[:], a2a_out.rearrange("(g e) s -> e g s", g=NC))

    # ------- W_O projection (attT), LN1 statistics interleaved per chunk -----
    ln1ctx = contextlib.ExitStack()
    ln1psum = ln1ctx.enter_context(tc.tile_pool(name="ln1psum", bufs=1, space="PSUM"))
    ln1ps1 = ln1psum.tile([1, SL], fp32, tag="ln1ps1", name="ln1ps1")
    ln1ps2 = ln1psum.tile([1, SL], fp32, tag="ln1ps2", name="ln1ps2")
    with tc.tile_pool(name="atpsum", bufs=2, space="PSUM") as atpsum, \
         tc.tile_pool(name="wosq", bufs=2) as wosq:
        for m in range(DK):
            ps = atpsum.tile([P, SL], fp32, tag="atps")
            for g in range(DK):
                nc.tensor.matmul(ps[:], wot_sb[:, g, ts(m, P)], ots_sb[:, g, :],
                                 start=(g == 0), stop=(g == DK - 1))
            nc.scalar.activation(att_sb[:, m, :], ps[:], Copy)
            sq = wosq.tile([P, SL], bf16, tag="sq")
            nc.vector.tensor_mul(sq[:], att_sb[:, m, :], att_sb[:, m, :])
            nc.tensor.matmul(ln1ps1[:], onesb_sb[:], att_sb[:, m, :],
                             start=(m == 0), stop=(m == DK - 1))
            nc.tensor.matmul(ln1ps2[:], onesb_sb[:], sq[:],
                             start=(m == 0), stop=(m == DK - 1))

    if stop_after == "wo":
        _ctx.close(); return

    # ------- LayerNorm application (stats psums precomputed upstream) --------
    def ln_apply(ps1, ps2, gi, bi, src_sb, res_sb, dsts, pools, out_cb=None,
                 res_b=None, dst_b=None):
        """dsts = res + gamma*(src-mean)/sqrt(var+eps) + beta.

        src_sb: [P, DK, SL] bf16; res_sb: [P, DK, SL] f32; ps1/ps2: [1, SL]
        sum / sum-of-squares psums; out_cb(g) runs after chunk g is written.
        """
        sqpool, stpool = pools
        mean = stpool.tile([1, SL], fp32, tag="mean")
        nc.vector.tensor_scalar_mul(mean[:], ps1[:], 1.0 / D)
        var = stpool.tile([1, SL], fp32, tag="var")
        nc.vector.tensor_scalar_mul(var[:], ps2[:], 1.0 / D)
        msq = stpool.tile([1, SL], fp32, tag="msq")
        nc.vector.tensor_mul(msq[:], mean[:], mean[:])
        nc.vector.tensor_tensor(var[:], var[:], msq[:], sub_op)
        nc.vector.tensor_scalar_add(var[:], var[:], EPS)
        std = stpool.tile([1, SL], fp32, tag="std")
        nc.scalar.activation(std[:], var[:], Sqrt)
        rstd = stpool.tile([1, SL], fp32, tag="rstd")
        nc.vector.reciprocal(rstd[:], std[:])
        mrs = stpool.tile([1, SL], fp32, tag="mrs")  # mean*rstd
        nc.vector.tensor_mul(mrs[:], mean[:], rstd[:])
        rstdh = stpool.tile([1, SL], bf16, tag="rstdh")
        nc.vector.tensor_copy(rstdh[:], rstd[:])
        mrsh = stpool.tile([1, SL], bf16, tag="mrsh")
        nc.vector.tensor_copy(mrsh[:], mrs[:])
        rstdb = stpool.tile([P, SL], bf16, tag="rstdb")
        nc.gpsimd.partition_broadcast(rstdb[:], rstdh[:])
        mrsb = stpool.tile([P, SL], bf16, tag="mrsb")
        nc.gpsimd.partition_broadcast(mrsb[:], mrsh[:])
        t1s = []
        for g in range(DK):
            t1 = sqpool.tile([P, SL], bf16, tag="t1", name="t1")
            nc.vector.tensor_mul(t1[:], src_sb[:, g, :], rstdb[:])
            nc.vector.tensor_tensor(t1[:], t1[:], mrsb[:], sub_op)
            nc.vector.tensor_scalar(t1[:], t1[:], gb_sb[:, gi, g:g + 1],
                                    gb_sb[:, bi, g:g + 1], mult_op, add_op)
            if dst_b is not None:   # fast bf16 residual add -> consumers start
                nc.vector.tensor_tensor(dst_b[:, g, :], t1[:], res_b[:, g, :],
                                        add_op)
                t1s.append(t1)
            else:
                dst0 = dsts[0]
                nc.vector.tensor_tensor(dst0[:, g, :], t1[:], res_sb[:, g, :],
                                        add_op)
                for dst in dsts[1:]:
                    nc.scalar.activation(dst[:, g, :], dst0[:, g, :], Copy)
            if out_cb is not None:
                out_cb(g)
        if dst_b is not None:       # deferred f32 path (needed only at LN2)
            for g in range(DK):
                nc.vector.tensor_tensor(dsts[0][:, g, :], t1s[g][:],
                                        res_sb[:, g, :], add_op)

    with tc.tile_pool(name="sqpool", bufs=9) as sqpool, \
         tc.tile_pool(name="stpool", bufs=2) as stpool:
        ln_apply(ln1ps1, ln1ps2, 0, 1, att_sb, eres_sb,
                 [x1t_sb], (sqpool, stpool), res_b=eresb_sb, dst_b=x1b_sb)
    ln1ctx.close()

    if stop_after == "ln1":
        _ctx.close(); return

    # ---------------- MLP (L1/L2 streamed; LN2 stats inside MLP2 loop) -------
    MCG = 4                       # mc columns per streamed L1 chunk
    ln2psum = _ctx.enter_context(tc.tile_pool(name="ln2psum", bufs=1, space="PSUM"))
    ln2ps1 = ln2psum.tile([1, SL], fp32, tag="ln2ps1", name="ln2ps1")
    ln2ps2 = ln2psum.tile([1, SL], fp32, tag="ln2ps2", name="ln2ps2")
    with tc.tile_pool(name="hallpool", bufs=1) as hallpool, \
         tc.tile_pool(name="mlsq", bufs=2) as mlsq, \
         tc.tile_pool(name="hpsum", bufs=3, space="PSUM") as hpsum:
        ht_all = hallpool.tile([P, MK, SL], bf16, tag="ht_all")
        for mcg in range(MK // MCG):
            l1c = l1pool.tile([P, DK, MCG * P], bf16, tag="l1c")
            nc.sync.dma_start(
                l1c[:], io["l1t"][:, ts(mcg, MCG * P)].rearrange(
                    "(g p) m -> p g m", p=P))
            for mi in range(MCG):
                mc = mcg * MCG + mi
                psh = hpsum.tile([P, SL], fp32, tag="psh")
                for g in range(DK):
                    nc.tensor.matmul(psh[:], l1c[:, g, ts(mi, P)], x1b_sb[:, g, :],
                                     start=(g == 0), stop=(g == DK - 1))
                nc.scalar.activation(ht_all[:, mc, :], psh[:], Relu,
                                     bias=l1b_sb[:, mc:mc + 1])
        with tc.tile_pool(name="mlppsum", bufs=2, space="PSUM") as mlppsum:
            for dt in range(DK):
                l2td = l2pool.tile([P, MK, P], bf16, tag="l2td")
                nc.sync.dma_start(l2td[:], io["l2t"][dt])
                psm2 = mlppsum.tile([P, SL], fp32, tag="psm2")
                for mc in range(MK):
                    nc.tensor.matmul(psm2[:], l2td[:, mc, :], ht_all[:, mc, :],
                                     start=(mc == 0), stop=(mc == MK - 1))
                nc.vector.tensor_scalar_add(mlp_sb[:, dt, :], psm2[:],
                                            gb_sb[:, 4, dt:dt + 1])
                sq = mlsq.tile([P, SL], bf16, tag="sq")
                nc.vector.tensor_mul(sq[:], mlp_sb[:, dt, :], mlp_sb[:, dt, :])
                nc.tensor.matmul(ln2ps1[:], onesb_sb[:], mlp_sb[:, dt, :],
                                 start=(dt == 0), stop=(dt == DK - 1))
                nc.tensor.matmul(ln2ps2[:], onesb_sb[:], sq[:],
                                 start=(dt == 0), stop=(dt == DK - 1))

    if stop_after == "mlp":
        _ctx.close(); return

    # ---------------- LN2 + residual; output stays [d, s] (host re-lays out) ----
    with tc.tile_pool(name="sqpool2", bufs=3) as sqpool2, \
         tc.tile_pool(name="stpool2", bufs=2) as stpool2, \
         tc.tile_pool(name="outpool", bufs=1) as outpool:
        outT = outpool.tile([P, DK, SL], fp32, tag="outT")
        outdram = io["out"].rearrange("(g p) s -> p g s", p=P)

        def flush(g):
            if g % 2 == 1:     # stream result out two chunks at a time
                nc.scalar.dma_start(outdram[:, g - 1:g + 1, :],
                                    outT[:, g - 1:g + 1, :])

        ln_apply(ln2ps1, ln2ps2, 2, 3, mlp_sb, x1t_sb, [outT],
                 (sqpool2, stpool2), out_cb=flush)
    _ctx.close()


def build_program(S=S_FULL, use_collective=True, stop_after=None):
    import concourse.mybir as mybir
    import concourse.tile as tile
    from concourse import bacc

    nc = bacc.Bacc("TRN2", target_bir_lowering=False, debug=False,
                   enable_asserts=True, num_devices=NC if use_collective else 1)
    f32, bf16 = mybir.dt.float32, mybir.dt.bfloat16

    def din(name, shape, dt=bf16):
        return nc.dram_tensor(name, shape, dt, kind="ExternalInput").ap()

    io = {
        "eT": din("eT", [D, S]),
        "eresT": din("eresT", [D, S // NC], f32),
        "eresb": din("eresb", [D, S // NC]),
        "wq": din("wq", [D, D]), "wk": din("wk", [D, D]),
        "wv": din("wv", [D, D]),
        "htq": din("htq", [D, EPC]), "htk": din("htk", [D, EPC]),
        "htv": din("htv", [D, EPC]),
        "wot": din("wot", [D, D]),
        "l1t": din("l1t", [D, DM]), "l2t": din("l2t", [DK, P, MK, P]),
        "l1b": din("l1b", [DM], f32), "l2b": din("l2b", [D], f32),
        "g1": din("g1", [D], f32), "b1": din("b1", [D], f32),
        "g2": din("g2", [D], f32), "b2": din("b2", [D], f32),
        "out": nc.dram_tensor("out", [D, S // NC], f32, kind="ExternalOutput").ap(),
    }
    with tile.TileContext(nc) as tc:
        _body(tc, io, S, use_collective, stop_after)
    nc.compile()
    return nc


def make_in_maps(E, mask, W_Q, W_K, W_V, W_O, H_Q, H_K, H_V,
                 L1_w, L1_b, L2_w, L2_b, gamma1, beta1, gamma2, beta2, S=S_FULL):
    E = np.asarray(E, np.float32).reshape(S, D)
    SL = S // NC
    com = {
        "eT": np.ascontiguousarray(E.T).astype(BF16),
        "wq": np.asarray(W_Q, np.float32).astype(BF16),
        "wk": np.asarray(W_K, np.float32).astype(BF16),
        "wv": np.asarray(W_V, np.float32).astype(BF16),
        "wot": np.ascontiguousarray(np.asarray(W_O, np.float32).T).astype(BF16),
        "l1t": np.ascontiguousarray(np.asarray(L1_w, np.float32).T).astype(BF16),
        "l2t": np.ascontiguousarray(
            np.asarray(L2_w, np.float32).T.reshape(MK, P, DK, P).transpose(2, 1, 0, 3)
        ).astype(BF16),
        "l1b": np.asarray(L1_b, np.float32), "l2b": np.asarray(L2_b, np.float32),
        "g1": np.asarray(gamma1, np.float32), "b1": np.asarray(beta1, np.float32),
        "g2": np.asarray(gamma2, np.float32), "b2": np.asarray(beta2, np.float32),
    }
    in_maps = []
    for c in range(NC):
        m = dict(com)
        m["eresT"] = np.ascontiguousarray(E[c * SL:(c + 1) * SL, :].T)
        m["eresb"] = m["eresT"].astype(BF16)
        for key, Hw in (("htq", H_Q), ("htk", H_K), ("htv", H_V)):
            hs = np.asarray(Hw, np.float32)[c * HPC:(c + 1) * HPC].reshape(EPC, D)
            m[key] = np.ascontiguousarray(hs.T).astype(BF16)
        in_maps.append(m)
    return in_maps


_PROGRAM_CACHE = {}


def kernel(**inputs):
    from concourse import bass_utils
    S = inputs["E"].shape[1]
    if S not in _PROGRAM_CACHE:
        _PROGRAM_CACHE[S] = build_program(S)
    nc = _PROGRAM_CACHE[S]
    in_maps = make_in_maps(S=S, **inputs)
    res = bass_utils.run_bass_kernel_spmd(nc, in_maps, core_ids=list(range(NC)))
    SL = S // NC
    out = np.concatenate([np.ascontiguousarray(res.results[c]["out"].T)
                          for c in range(NC)], axis=0)
    return out.reshape(1, S, D).astype(np.float32)


# revision 30
# speedup vs baseline: 1.0018x; 1.0018x over previous
"""Trainium2 Bass kernel for nn_DecoderBlock (B=1, S=2048, D=1024, H=16, DQ=64, DM=4096).

Strategy (8 NeuronCores, one chip):
  - Attention tensor-parallel over heads: core c owns heads {2c, 2c+1}.
    Per-head projections fused with QKV: weffT[din, e] = (H_x[h] @ W_x).T,
    computed on device transpose-free by contracting W chunks against H.T.
    Scores are computed TRANSPOSED ([keys, queries]) so softmax's reduction
    lands on the matmul contraction axis: a row of ones appended to v gives
    the softmax denominators for free.
  - Causality exploited: key-tiles strictly above the diagonal are skipped
    (never computed); diagonal tiles are masked in-place with a device-side
    affine_select on the GpSimd engine (no mask tensor, no mask DMA at all).
  - One AllToAll exchanges attention outputs (heads-sharded -> token-sharded).
  - Post-attention (W_O, LN1, MLP, LN2, residuals) data-parallel over tokens:
    core c owns tokens [c*256, (c+1)*256) in transposed [d, s] layout;
    LayerNorm statistics via ones-matmuls over the partition axis.
  - All weight/activation DMA payloads are bf16 (half the HBM traffic of
    f32/f32r); matmuls run bf16 with fp32 PSUM accumulation; the two
    residual adds and LN statistics chains stay fp32.
  - MLP weights (L1 full, L2 streamed) prefetch during attention so the DMA
    engines and PE overlap instead of serializing.
"""

import numpy as np
import ml_dtypes

BF16 = ml_dtypes.bfloat16

B, S_FULL, D, H, DQ, DM = 1, 2048, 1024, 16, 64, 4096
NC = 8          # cores
P = 128         # partitions
HPC = H // NC   # heads per core
EPC = HPC * DQ  # per-core attention width (128)
DK = D // P     # d-model chunks (8)
MK = DM // P    # mlp chunks (32)
EPS = 1e-5


def _body(tc, io, S, use_collective=True, stop_after=None):
    import concourse.bass as bass
    import concourse.mybir as mybir
    from concourse.masks import make_identity

    nc = tc.nc
    fp32 = mybir.dt.float32
    bf16 = mybir.dt.bfloat16
    Exp = mybir.ActivationFunctionType.Exp
    Relu = mybir.ActivationFunctionType.Relu
    Sqrt = mybir.ActivationFunctionType.Sqrt
    Copy = mybir.ActivationFunctionType.Copy
    sub_op = mybir.AluOpType.subtract
    mult_op = mybir.AluOpType.mult
    add_op = mybir.AluOpType.add
    ge_op = mybir.AluOpType.is_ge

    SL = S // NC                 # tokens per core (256)
    TT = S // P                  # key tiles (16)
    SPW = 512                    # attention query-strip width
    NSP = S // SPW               # 4
    TPS = SPW // P               # key tiles per strip width (4)
    QH = min(1024, S)            # qkv streaming half width
    NQH = S // QH                # 2
    QSP = QH // SPW              # strips per half (2)

    ts = bass.ts

    # ---------------- persistent SBUF ----------------
    import contextlib
    _ctx = contextlib.ExitStack()
    persist = _ctx.enter_context(tc.tile_pool(name="persist", bufs=1))

    def ptile(shape, dt, tag):
        return persist.tile(shape, dt, tag=tag, name=tag)

    drampool = _ctx.enter_context(tc.tile_pool(name="drampool", bufs=1, space="DRAM"))

    qt_sb = ptile([EPC, S], bf16, "qt_sb")
    kt_sb = ptile([EPC, S], bf16, "kt_sb")
    vaug_sb = ptile([P, HPC, TT, DQ + 1], bf16, "vaug_sb")
    ot_sb = ptile([EPC, S], bf16, "ot_sb")            # attn out (2 heads)
    weff_sb = ptile([P, 3, DK, EPC], bf16, "weff_sb")  # weffT[din_p, j, kk, e]
    onesb_sb = ptile([P, 1], bf16, "onesb_sb")
    gb_sb = ptile([P, 6, DK], fp32, "gb_sb")          # g1,b1,g2,b2,l2b,(pad)
    l1b_sb = ptile([P, MK], fp32, "l1b_sb")
    wot_sb = ptile([P, DK, D], bf16, "wot_sb")
    eres_sb = ptile([P, DK, SL], fp32, "eres_sb")
    eresb_sb = ptile([P, DK, SL], bf16, "eresb_sb")
    x1t_sb = ptile([P, DK, SL], fp32, "x1t_sb")
    x1b_sb = ptile([P, DK, SL], bf16, "x1b_sb")
    att_sb = ptile([P, DK, SL], bf16, "att_sb")
    mlp_sb = ptile([P, DK, SL], bf16, "mlp_sb")
    ots_sb = ptile([P, DK, SL], bf16, "ots_sb")       # o for my tokens

    tri_sb = ptile([P, TPS, SPW], bf16, "tri_sb")     # diagonal-tile causal masks

    # eT in strip-chunk DMAs on the ACT queue (sync queue carries weights);
    # qkv strip sp only needs chunk sp, so compute starts before eT finishes
    l1pool = _ctx.enter_context(tc.tile_pool(name="l1pool", bufs=2))
    l2pool = _ctx.enter_context(tc.tile_pool(name="l2pool", bufs=2))
    etctx = contextlib.ExitStack()
    etpool = etctx.enter_context(tc.tile_pool(name="etpool", bufs=1))
    et_sb = etpool.tile([P, DK, S], bf16, tag="et_sb", name="et_sb")

    nc.gpsimd.memset(onesb_sb[:], 1.0)
    nc.gpsimd.memset(vaug_sb[:, :, :, DQ:], 1.0)      # denominator rows
    nc.gpsimd.memset(tri_sb[:], 1.0)
    for j in range(TPS):   # keep (1.0) iff query >= key inside offset-j tile
        nc.gpsimd.affine_select(
            out=tri_sb[:, j, :], in_=tri_sb[:, j, :], compare_op=ge_op,
            fill=0.0, base=-P * j, channel_multiplier=-1, pattern=[[1, SPW]])

    # ---------------- weffT[j] = sum_dout W_chunk.T-contraction vs H.T --------
    with tc.tile_pool(name="htpool", bufs=1) as htpool, \
         tc.tile_pool(name="wpool", bufs=3) as wpool, \
         tc.tile_pool(name="wpsum", bufs=2, space="PSUM") as wpsum:
        ht = htpool.tile([P, 3, DK, EPC], bf16, tag="ht")
        for j, hn in enumerate(("htq", "htk", "htv")):
            nc.sync.dma_start(ht[:, j], io[hn].rearrange("(dh p) e -> p dh e", p=P))
        w_tiles = []
        for j, wn in enumerate(("wq", "wk", "wv")):
            w_all = wpool.tile([P, DK, D], bf16, tag="w_all", name="w_all")
            nc.sync.dma_start(w_all[:], io[wn].rearrange("(dh p) e -> p dh e", p=P))
            w_tiles.append(w_all)
            if j == 0:          # eT strip 0 right after wq, rest after wv
                nc.sync.dma_start(
                    et_sb[:, :, ts(0, SPW)],
                    io["eT"][:, ts(0, SPW)].rearrange("(kk p) s -> p kk s", p=P))
        for sp in range(1, S // SPW):
            nc.sync.dma_start(
                et_sb[:, :, ts(sp, SPW)],
                io["eT"][:, ts(sp, SPW)].rearrange("(kk p) s -> p kk s", p=P))
        for j in range(3):
            w_all = w_tiles[j]
            psT = wpsum.tile([P, DK, EPC], fp32, tag="psT")
            for kk in range(DK):
                for dh in range(DK):
                    nc.tensor.matmul(psT[:, kk, :], w_all[:, dh, ts(kk, P)],
                                     ht[:, j, dh, :],
                                     start=(dh == 0), stop=(dh == DK - 1))
            if j == 0:  # fold 1/sqrt(DQ) into q weights
                nc.scalar.activation(weff_sb[:, j], psT[:], Copy,
                                     scale=1.0 / float(np.sqrt(DQ)))
            else:
                nc.scalar.activation(weff_sb[:, j], psT[:], Copy)

    # -------- prefetch W_O + small tensors (DMA overlaps attention) ----------
    nc.sync.dma_start(wot_sb[:], io["wot"].rearrange("(g p) e -> p g e", p=P))
    for i, name in enumerate(("g1", "b1", "g2", "b2", "l2b")):
        nc.sync.dma_start(gb_sb[:, i, :], io[name].rearrange("(g p) -> p g", p=P))
    nc.sync.dma_start(l1b_sb[:], io["l1b"].rearrange("(g p) -> p g", p=P))
    nc.sync.dma_start(eres_sb[:], io["eresT"].rearrange("(g p) s -> p g s", p=P))
    nc.sync.dma_start(eresb_sb[:], io["eresb"].rearrange("(g p) s -> p g s", p=P))

    a2a_in = drampool.tile([NC * EPC, SL], bf16, tag="a2a_in", name="a2a_in")
    a2a_out = drampool.tile([NC * EPC, SL], bf16, tag="a2a_out", name="a2a_out")

    # ------- fused per-strip qkv projection + causal attention ---------------
    # Strip sp: project q/k for queries in the strip and v for its key tiles
    # (v computed directly in [keys, e] layout: eT-tile stationary, weffT
    # moving -- no transposes), then run attention for key tiles 0..4sp+3.
    # exp runs on ACT (the strip-phase bottleneck), all copies/masks on DVE,
    # o-matmuls lag score-matmuls by GD tiles so PE/ACT/DVE pipeline.
    GD = 2
    with tc.tile_pool(name="expool", bufs=6) as expool, \
         tc.tile_pool(name="scpsum", bufs=2, space="PSUM") as scpsum, \
         tc.tile_pool(name="opsum", bufs=1, space="PSUM") as opsum, \
         tc.tile_pool(name="qkpsum", bufs=1, space="PSUM") as qkpsum, \
         tc.tile_pool(name="pvpsum", bufs=1, space="PSUM") as pvpsum, \
         tc.tile_pool(name="nrmpool", bufs=2) as nrmpool:
        carry = None          # (sp, tmax, pso, exs) pending from previous strip

        def finish_strip(sp0, tmax0, pso0, exs0):
            qs0 = slice(sp0 * SPW, (sp0 + 1) * SPW)
            for t2 in range(tmax0 - GD, tmax0):
                ex2 = exs0.pop(t2)
                for h in range(HPC):
                    nc.tensor.matmul(pso0[h][:], vaug_sb[:, h, t2, :],
                                     ex2[:, h, :],
                                     start=(t2 == 0), stop=(t2 == tmax0 - 1))
            for h in range(HPC):
                hs = slice(h * DQ, (h + 1) * DQ)
                rc = nrmpool.tile([1, SPW], fp32, tag="rc")
                nc.vector.reciprocal(rc[:], pso0[h][DQ:DQ + 1, :])
                rcb = nrmpool.tile([DQ, SPW], fp32, tag="rcb")
                nc.gpsimd.partition_broadcast(rcb[:], rc[:], channels=DQ)
                nc.vector.tensor_mul(ot_sb[hs, qs0], pso0[h][:DQ, :], rcb[:])
            # stage this strip's heads-block into the AllToAll source buffer
            # (gpsimd queue: the ACT queue must keep streaming exps)
            nc.gpsimd.dma_start(
                a2a_in[HPC * sp0 * EPC:HPC * (sp0 + 1) * EPC, :].rearrange(
                    "(g e) s -> e g s", g=HPC),
                ot_sb[:, qs0].rearrange("e (g s) -> e g s", g=HPC))

        def emit_qkv(sp):
            qs = slice(sp * SPW, (sp + 1) * SPW)
            for j, dst in ((0, qt_sb), (1, kt_sb)):
                pj = qkpsum.tile([EPC, SPW], fp32, tag="pj", name="pj")
                for kk in range(DK):
                    nc.tensor.matmul(pj[:], weff_sb[:, j, kk, :],
                                     et_sb[:, kk, qs],
                                     start=(kk == 0), stop=(kk == DK - 1))
                nc.vector.tensor_copy(dst[:, qs], pj[:])
            for ti in range(TPS):
                t = TPS * sp + ti
                pv = pvpsum.tile([P, EPC], fp32, tag="pv", name="pv")
                for kk in range(DK):
                    nc.tensor.matmul(pv[:], et_sb[:, kk, ts(t, P)],
                                     weff_sb[:, 2, kk, :],
                                     start=(kk == 0), stop=(kk == DK - 1))
                nc.vector.tensor_copy(vaug_sb[:, :, t, :DQ],
                                      pv.rearrange("p (h e) -> p h e", h=HPC))

        for sp in range(NSP):
            qs = slice(sp * SPW, (sp + 1) * SPW)
            emit_qkv(sp)
            if carry is not None:
                finish_strip(*carry)
            tmax = TPS * (sp + 1)          # key tiles needed by this strip
            pso = [opsum.tile([DQ + 1, SPW], fp32, tag=f"pso{h}", name=f"pso{h}")
                   for h in range(HPC)]
            exs = {}
            for tt in range(tmax):
                if tt < tmax:
                    t = tt
                    ps = scpsum.tile([P, HPC, SPW], fp32, tag="scps")
                    for h in range(HPC):
                        hs = slice(h * DQ, (h + 1) * DQ)
                        nc.tensor.matmul(ps[:, h, :], kt_sb[hs, ts(t, P)],
                                         qt_sb[hs, qs], start=True, stop=True)
                    ex = expool.tile([P, HPC, SPW], bf16, tag="ex")
                    nc.scalar.activation(ex[:], ps[:], Exp)
                    if t >= TPS * sp:      # diagonal tile: causal mask in-place
                        for h in range(HPC):
                            nc.vector.tensor_mul(ex[:, h, :], ex[:, h, :],
                                                 tri_sb[:, t - TPS * sp, :])
                    exs[t] = ex
                if tt >= GD:
                    t2 = tt - GD
                    ex2 = exs.pop(t2)
                    for h in range(HPC):
                        nc.tensor.matmul(pso[h][:], vaug_sb[:, h, t2, :],
                                         ex2[:, h, :],
                                         start=(t2 == 0), stop=(t2 == tmax - 1))
            carry = (sp, tmax, pso, exs)
        finish_strip(*carry)
    etctx.close()

    if stop_after in ("qkv", "attn"):
        _ctx.close(); return

    # ---------------- AllToAll: heads-sharded -> token-sharded ----------------
    a2a_in = drampool.tile([NC * EPC, SL], bf16, tag="a2a_in", name="a2a_in")
    a2a_out = drampool.tile([NC * EPC, SL], bf16, tag="a2a_out", name="a2a_out")
    nc.scalar.dma_start(a2a_in.rearrange("(g e) s -> e g s", g=NC),
                        ot_sb.rearrange("e (g s) -> e g s", g=NC))
    if use_collective:
        nc.gpsimd.collective_compute(
            "AllToAll", mybir.AluOpType.bypass,
            replica_groups=[list(range(NC))],
            ins=[a2a_in.opt()], outs=[a2a_out.opt()])
    else:  # timing-only single-core variant (results wrong across cores)
        nc.scalar.dma_start(a2a_out[:], a2a_in[:])
    nc.scalar.dma_start(ots_sb[:], a2a_out.rearrange("(g e) s -> e g s", g=NC))

    # ------- W_O projection (attT), LN1 statistics interleaved per chunk -----
    ln1ctx = contextlib.ExitStack()
    ln1psum = ln1ctx.enter_context(tc.tile_pool(name="ln1psum", bufs=1, space="PSUM"))
    ln1ps1 = ln1psum.tile([1, SL], fp32, tag="ln1ps1", name="ln1ps1")
    ln1ps2 = ln1psum.tile([1, SL], fp32, tag="ln1ps2", name="ln1ps2")
    with tc.tile_pool(name="atpsum", bufs=2, space="PSUM") as atpsum, \
         tc.tile_pool(name="wosq", bufs=2) as wosq:
        for m in range(DK):
            ps = atpsum.tile([P, SL], fp32, tag="atps")
            for g in range(DK):
                nc.tensor.matmul(ps[:], wot_sb[:, g, ts(m, P)], ots_sb[:, g, :],
                                 start=(g == 0), stop=(g == DK - 1))
            nc.scalar.activation(att_sb[:, m, :], ps[:], Copy)
            sq = wosq.tile([P, SL], bf16, tag="sq")
            nc.vector.tensor_mul(sq[:], att_sb[:, m, :], att_sb[:, m, :])
            nc.tensor.matmul(ln1ps1[:], onesb_sb[:], att_sb[:, m, :],
                             start=(m == 0), stop=(m == DK - 1))
            nc.tensor.matmul(ln1ps2[:], onesb_sb[:], sq[:],
                             start=(m == 0), stop=(m == DK - 1))

    if stop_after == "wo":
        _ctx.close(); return

    # ------- LayerNorm application (stats psums precomputed upstream) --------
    def ln_apply(ps1, ps2, gi, bi, src_sb, res_sb, dsts, pools, out_cb=None,
                 res_b=None, dst_b=None):
        """dsts = res + gamma*(src-mean)/sqrt(var+eps) + beta.

        src_sb: [P, DK, SL] bf16; res_sb: [P, DK, SL] f32; ps1/ps2: [1, SL]
        sum / sum-of-squares psums; out_cb(g) runs after chunk g is written.
        """
        sqpool, stpool = pools
        mean = stpool.tile([1, SL], fp32, tag="mean")
        nc.vector.tensor_scalar_mul(mean[:], ps1[:], 1.0 / D)
        var = stpool.tile([1, SL], fp32, tag="var")
        nc.vector.tensor_scalar_mul(var[:], ps2[:], 1.0 / D)
        msq = stpool.tile([1, SL], fp32, tag="msq")
        nc.vector.tensor_mul(msq[:], mean[:], mean[:])
        nc.vector.tensor_tensor(var[:], var[:], msq[:], sub_op)
        nc.vector.tensor_scalar_add(var[:], var[:], EPS)
        std = stpool.tile([1, SL], fp32, tag="std")
        nc.scalar.activation(std[:], var[:], Sqrt)
        rstd = stpool.tile([1, SL], fp32, tag="rstd")
        nc.vector.reciprocal(rstd[:], std[:])
        mrs = stpool.tile([1, SL], fp32, tag="mrs")  # mean*rstd
        nc.vector.tensor_mul(mrs[:], mean[:], rstd[:])
        rstdh = stpool.tile([1, SL], bf16, tag="rstdh")
        nc.vector.tensor_copy(rstdh[:], rstd[:])
        mrsh = stpool.tile([1, SL], bf16, tag="mrsh")
        nc.vector.tensor_copy(mrsh[:], mrs[:])
        rstdb = stpool.tile([P, SL], bf16, tag="rstdb")
        nc.gpsimd.partition_broadcast(rstdb[:], rstdh[:])
        mrsb = stpool.tile([P, SL], bf16, tag="mrsb")
        nc.gpsimd.partition_broadcast(mrsb[:], mrsh[:])
        t1s = []
        for g in range(DK):
            t1 = sqpool.tile([P, SL], bf16, tag="t1", name="t1")
            nc.vector.tensor_mul(t1[:], src_sb[:, g, :], rstdb[:])
            nc.vector.tensor_tensor(t1[:], t1[:], mrsb[:], sub_op)
            nc.vector.tensor_scalar(t1[:], t1[:], gb_sb[:, gi, g:g + 1],
                                    gb_sb[:, bi, g:g + 1], mult_op, add_op)
            if dst_b is not None:   # fast bf16 residual add -> consumers start
                nc.vector.tensor_tensor(dst_b[:, g, :], t1[:], res_b[:, g, :],
                                        add_op)
                t1s.append(t1)
            else:
                dst0 = dsts[0]
                nc.vector.tensor_tensor(dst0[:, g, :], t1[:], res_sb[:, g, :],
                                        add_op)
                for dst in dsts[1:]:
                    nc.scalar.activation(dst[:, g, :], dst0[:, g, :], Copy)
            if out_cb is not None:
                out_cb(g)
        if dst_b is not None:       # deferred f32 path (needed only at LN2)
            for g in range(DK):
                nc.vector.tensor_tensor(dsts[0][:, g, :], t1s[g][:],
                                        res_sb[:, g, :], add_op)

    with tc.tile_pool(name="sqpool", bufs=9) as sqpool, \
         tc.tile_pool(name="stpool", bufs=2) as stpool:
        ln_apply(ln1ps1, ln1ps2, 0, 1, att_sb, eres_sb,
                 [x1t_sb], (sqpool, stpool), res_b=eresb_sb, dst_b=x1b_sb)
    ln1ctx.close()

    if stop_after == "ln1":
        _ctx.close(); return

    # ---------------- MLP (L1/L2 streamed; LN2 stats inside MLP2 loop) -------
    MCG = 4                       # mc columns per streamed L1 chunk
    ln2psum = _ctx.enter_context(tc.tile_pool(name="ln2psum", bufs=1, space="PSUM"))
    ln2ps1 = ln2psum.tile([1, SL], fp32, tag="ln2ps1", name="ln2ps1")
    ln2ps2 = ln2psum.tile([1, SL], fp32, tag="ln2ps2", name="ln2ps2")
    with tc.tile_pool(name="hallpool", bufs=1) as hallpool, \
         tc.tile_pool(name="mlsq", bufs=2) as mlsq, \
         tc.tile_pool(name="hpsum", bufs=3, space="PSUM") as hpsum:
        ht_all = hallpool.tile([P, MK, SL], bf16, tag="ht_all")
        for mcg in range(MK // MCG):
            l1c = l1pool.tile([P, DK, MCG * P], bf16, tag="l1c")
            nc.sync.dma_start(
                l1c[:], io["l1t"][:, ts(mcg, MCG * P)].rearrange(
                    "(g p) m -> p g m", p=P))
            for mi in range(MCG):
                mc = mcg * MCG + mi
                psh = hpsum.tile([P, SL], fp32, tag="psh")
                for g in range(DK):
                    nc.tensor.matmul(psh[:], l1c[:, g, ts(mi, P)], x1b_sb[:, g, :],
                                     start=(g == 0), stop=(g == DK - 1))
                nc.scalar.activation(ht_all[:, mc, :], psh[:], Relu,
                                     bias=l1b_sb[:, mc:mc + 1])
        with tc.tile_pool(name="mlppsum", bufs=2, space="PSUM") as mlppsum:
            for dt in range(DK):
                l2td = l2pool.tile([P, MK, P], bf16, tag="l2td")
                nc.sync.dma_start(l2td[:], io["l2t"][dt])
                psm2 = mlppsum.tile([P, SL], fp32, tag="psm2")
                for mc in range(MK):
                    nc.tensor.matmul(psm2[:], l2td[:, mc, :], ht_all[:, mc, :],
                                     start=(mc == 0), stop=(mc == MK - 1))
                nc.vector.tensor_scalar_add(mlp_sb[:, dt, :], psm2[:],
                                            gb_sb[:, 4, dt:dt + 1])
                sq = mlsq.tile([P, SL], bf16, tag="sq")
                nc.vector.tensor_mul(sq[:], mlp_sb[:, dt, :], mlp_sb[:, dt, :])
                nc.tensor.matmul(ln2ps1[:], onesb_sb[:], mlp_sb[:, dt, :],
                                 start=(dt == 0), stop=(dt == DK - 1))
                nc.tensor.matmul(ln2ps2[:], onesb_sb[:], sq[:],
                                 start=(dt == 0), stop=(dt == DK - 1))

    if stop_after == "mlp":
        _ctx.close(); return

    # ---------------- LN2 + residual; output stays [d, s] (host re-lays out) ----
    with tc.tile_pool(name="sqpool2", bufs=3) as sqpool2, \
         tc.tile_pool(name="stpool2", bufs=2) as stpool2, \
         tc.tile_pool(name="outpool", bufs=1) as outpool:
        outT = outpool.tile([P, DK, SL], fp32, tag="outT")
        outdram = io["out"].rearrange("(g p) s -> p g s", p=P)

        def flush(g):
            if g % 2 == 1:     # stream result out two chunks at a time
                nc.scalar.dma_start(outdram[:, g - 1:g + 1, :],
                                    outT[:, g - 1:g + 1, :])

        ln_apply(ln2ps1, ln2ps2, 2, 3, mlp_sb, x1t_sb, [outT],
                 (sqpool2, stpool2), out_cb=flush)
    _ctx.close()


def build_program(S=S_FULL, use_collective=True, stop_after=None):
    import concourse.mybir as mybir
    import concourse.tile as tile
    from concourse import bacc

    nc = bacc.Bacc("TRN2", target_bir_lowering=False, debug=False,
                   enable_asserts=True, num_devices=NC if use_collective else 1)
    f32, bf16 = mybir.dt.float32, mybir.dt.bfloat16

    def din(name, shape, dt=bf16):
        return nc.dram_tensor(name, shape, dt, kind="ExternalInput").ap()

    io = {
        "eT": din("eT", [D, S]),
        "eresT": din("eresT", [D, S // NC], f32),
        "eresb": din("eresb", [D, S // NC]),
        "wq": din("wq", [D, D]), "wk": din("wk", [D, D]),
        "wv": din("wv", [D, D]),
        "htq": din("htq", [D, EPC]), "htk": din("htk", [D, EPC]),
        "htv": din("htv", [D, EPC]),
        "wot": din("wot", [D, D]),
        "l1t": din("l1t", [D, DM]), "l2t": din("l2t", [DK, P, MK, P]),
        "l1b": din("l1b", [DM], f32), "l2b": din("l2b", [D], f32),
        "g1": din("g1", [D], f32), "b1": din("b1", [D], f32),
        "g2": din("g2", [D], f32), "b2": din("b2", [D], f32),
        "out": nc.dram_tensor("out", [D, S // NC], f32, kind="ExternalOutput").ap(),
    }
    with tile.TileContext(nc) as tc:
        _body(tc, io, S, use_collective, stop_after)
    nc.compile()
    return nc


def make_in_maps(E, mask, W_Q, W_K, W_V, W_O, H_Q, H_K, H_V,
                 L1_w, L1_b, L2_w, L2_b, gamma1, beta1, gamma2, beta2, S=S_FULL):
    E = np.asarray(E, np.float32).reshape(S, D)
    SL = S // NC
    com = {
        "eT": np.ascontiguousarray(E.T).astype(BF16),
        "wq": np.asarray(W_Q, np.float32).astype(BF16),
        "wk": np.asarray(W_K, np.float32).astype(BF16),
        "wv": np.asarray(W_V, np.float32).astype(BF16),
        "wot": np.ascontiguousarray(np.asarray(W_O, np.float32).T).astype(BF16),
        "l1t": np.ascontiguousarray(np.asarray(L1_w, np.float32).T).astype(BF16),
        "l2t": np.ascontiguousarray(
            np.asarray(L2_w, np.float32).T.reshape(MK, P, DK, P).transpose(2, 1, 0, 3)
        ).astype(BF16),
        "l1b": np.asarray(L1_b, np.float32), "l2b": np.asarray(L2_b, np.float32),
        "g1": np.asarray(gamma1, np.float32), "b1": np.asarray(beta1, np.float32),
        "g2": np.asarray(gamma2, np.float32), "b2": np.asarray(beta2, np.float32),
    }
    in_maps = []
    for c in range(NC):
        m = dict(com)
        m["eresT"] = np.ascontiguousarray(E[c * SL:(c + 1) * SL, :].T)
        m["eresb"] = m["eresT"].astype(BF16)
        for key, Hw in (("htq", H_Q), ("htk", H_K), ("htv", H_V)):
            hs = np.asarray(Hw, np.float32)[c * HPC:(c + 1) * HPC].reshape(EPC, D)
            m[key] = np.ascontiguousarray(hs.T).astype(BF16)
        in_maps.append(m)
    return in_maps


_PROGRAM_CACHE = {}


def kernel(**inputs):
    from concourse import bass_utils
    S = inputs["E"].shape[1]
    if S not in _PROGRAM_CACHE:
        _PROGRAM_CACHE[S] = build_program(S)
    nc = _PROGRAM_CACHE[S]
    in_maps = make_in_maps(S=S, **inputs)
    res = bass_utils.run_bass_kernel_spmd(nc, in_maps, core_ids=list(range(NC)))
    SL = S // NC
    out = np.concatenate([np.ascontiguousarray(res.results[c]["out"].T)
                          for c in range(NC)], axis=0)
    return out.reshape(1, S, D).astype(np.float32)


# revision 32
# speedup vs baseline: 1.0416x; 1.0397x over previous
"""Trainium2 Bass kernel for nn_DecoderBlock (B=1, S=2048, D=1024, H=16, DQ=64, DM=4096).

Strategy (8 NeuronCores, one chip):
  - Attention tensor-parallel over heads: core c owns heads {2c, 2c+1}.
    Per-head projections fused with QKV: weffT[din, e] = (H_x[h] @ W_x).T,
    computed on device transpose-free by contracting W chunks against H.T.
    Scores are computed TRANSPOSED ([keys, queries]) so softmax's reduction
    lands on the matmul contraction axis: a row of ones appended to v gives
    the softmax denominators for free.
  - Causality exploited: key-tiles strictly above the diagonal are skipped
    (never computed); diagonal tiles are masked in-place with a device-side
    affine_select on the GpSimd engine (no mask tensor, no mask DMA at all).
  - One AllToAll exchanges attention outputs (heads-sharded -> token-sharded).
  - Post-attention (W_O, LN1, MLP, LN2, residuals) data-parallel over tokens:
    core c owns tokens [c*256, (c+1)*256) in transposed [d, s] layout;
    LayerNorm statistics via ones-matmuls over the partition axis.
  - All weight/activation DMA payloads are bf16 (half the HBM traffic of
    f32/f32r); matmuls run bf16 with fp32 PSUM accumulation; the two
    residual adds and LN statistics chains stay fp32.
  - MLP weights (L1 full, L2 streamed) prefetch during attention so the DMA
    engines and PE overlap instead of serializing.
"""

import numpy as np
import ml_dtypes

BF16 = ml_dtypes.bfloat16

B, S_FULL, D, H, DQ, DM = 1, 2048, 1024, 16, 64, 4096
NC = 8          # cores
P = 128         # partitions
HPC = H // NC   # heads per core
EPC = HPC * DQ  # per-core attention width (128)
DK = D // P     # d-model chunks (8)
MK = DM // P    # mlp chunks (32)
EPS = 1e-5


def _body(tc, io, S, use_collective=True, stop_after=None):
    import concourse.bass as bass
    import concourse.mybir as mybir
    from concourse.masks import make_identity

    nc = tc.nc
    fp32 = mybir.dt.float32
    bf16 = mybir.dt.bfloat16
    Exp = mybir.ActivationFunctionType.Exp
    Relu = mybir.ActivationFunctionType.Relu
    Sqrt = mybir.ActivationFunctionType.Sqrt
    Copy = mybir.ActivationFunctionType.Copy
    sub_op = mybir.AluOpType.subtract
    mult_op = mybir.AluOpType.mult
    add_op = mybir.AluOpType.add
    ge_op = mybir.AluOpType.is_ge

    SL = S // NC                 # tokens per core (256)
    TT = S // P                  # key tiles (16)
    SPW = 512                    # attention query-strip width
    NSP = S // SPW               # 4
    TPS = SPW // P               # key tiles per strip width (4)
    QH = min(1024, S)            # qkv streaming half width
    NQH = S // QH                # 2
    QSP = QH // SPW              # strips per half (2)

    ts = bass.ts

    # ---------------- persistent SBUF ----------------
    import contextlib
    _ctx = contextlib.ExitStack()
    persist = _ctx.enter_context(tc.tile_pool(name="persist", bufs=1))

    def ptile(shape, dt, tag):
        return persist.tile(shape, dt, tag=tag, name=tag)

    drampool = _ctx.enter_context(tc.tile_pool(name="drampool", bufs=1, space="DRAM"))

    qt_sb = ptile([EPC, S], bf16, "qt_sb")
    kt_sb = ptile([EPC, S], bf16, "kt_sb")
    vaug_sb = ptile([P, HPC, TT, DQ + 1], bf16, "vaug_sb")
    ot_sb = ptile([EPC, S], bf16, "ot_sb")            # attn out (2 heads)
    weff_sb = ptile([P, 3, DK, EPC], bf16, "weff_sb")  # weffT[din_p, j, kk, e]
    onesb_sb = ptile([P, 1], bf16, "onesb_sb")
    gb_sb = ptile([P, 6, DK], fp32, "gb_sb")          # g1,b1,g2,b2,l2b,(pad)
    l1b_sb = ptile([P, MK], fp32, "l1b_sb")
    wot_sb = ptile([P, DK, D], bf16, "wot_sb")
    eres_sb = ptile([P, DK, SL], fp32, "eres_sb")
    eresb_sb = ptile([P, DK, SL], bf16, "eresb_sb")
    x1t_sb = ptile([P, DK, SL], fp32, "x1t_sb")
    x1b_sb = ptile([P, DK, SL], bf16, "x1b_sb")
    att_sb = ptile([P, DK, SL], bf16, "att_sb")
    mlp_sb = ptile([P, DK, SL], bf16, "mlp_sb")
    ots_sb = ptile([P, DK, SL], bf16, "ots_sb")       # o for my tokens

    tri_sb = ptile([P, TPS, SPW], bf16, "tri_sb")     # diagonal-tile causal masks

    # eT in strip-chunk DMAs on the ACT queue (sync queue carries weights);
    # qkv strip sp only needs chunk sp, so compute starts before eT finishes
    l1pool = _ctx.enter_context(tc.tile_pool(name="l1pool", bufs=2))
    l2pool = _ctx.enter_context(tc.tile_pool(name="l2pool", bufs=3))
    etctx = contextlib.ExitStack()
    etpool = etctx.enter_context(tc.tile_pool(name="etpool", bufs=1))
    et_sb = etpool.tile([P, DK, S], bf16, tag="et_sb", name="et_sb")

    nc.gpsimd.memset(onesb_sb[:], 1.0)
    nc.gpsimd.memset(vaug_sb[:, :, :, DQ:], 1.0)      # denominator rows
    nc.gpsimd.memset(tri_sb[:], 1.0)
    for j in range(TPS):   # keep (1.0) iff query >= key inside offset-j tile
        nc.gpsimd.affine_select(
            out=tri_sb[:, j, :], in_=tri_sb[:, j, :], compare_op=ge_op,
            fill=0.0, base=-P * j, channel_multiplier=-1, pattern=[[1, SPW]])

    # ---------------- weffT[j] = sum_dout W_chunk.T-contraction vs H.T --------
    with tc.tile_pool(name="htpool", bufs=1) as htpool, \
         tc.tile_pool(name="wpool", bufs=3) as wpool, \
         tc.tile_pool(name="wpsum", bufs=2, space="PSUM") as wpsum:
        ht = htpool.tile([P, 3, DK, EPC], bf16, tag="ht")
        for j, hn in enumerate(("htq", "htk", "htv")):
            nc.sync.dma_start(ht[:, j], io[hn].rearrange("(dh p) e -> p dh e", p=P))
        w_tiles = []
        for j, wn in enumerate(("wq", "wk", "wv")):
            w_all = wpool.tile([P, DK, D], bf16, tag="w_all", name="w_all")
            nc.sync.dma_start(w_all[:], io[wn].rearrange("(dh p) e -> p dh e", p=P))
            w_tiles.append(w_all)
            if j == 0:          # eT strip 0 right after wq, rest after wv
                nc.sync.dma_start(
                    et_sb[:, :, ts(0, SPW)],
                    io["eT"][:, ts(0, SPW)].rearrange("(kk p) s -> p kk s", p=P))
        for sp in range(1, S // SPW):
            nc.sync.dma_start(
                et_sb[:, :, ts(sp, SPW)],
                io["eT"][:, ts(sp, SPW)].rearrange("(kk p) s -> p kk s", p=P))
        for j in range(3):
            w_all = w_tiles[j]
            psT = wpsum.tile([P, DK, EPC], fp32, tag="psT")
            for kk in range(DK):
                for dh in range(DK):
                    nc.tensor.matmul(psT[:, kk, :], w_all[:, dh, ts(kk, P)],
                                     ht[:, j, dh, :],
                                     start=(dh == 0), stop=(dh == DK - 1))
            if j == 0:  # fold 1/sqrt(DQ) into q weights
                nc.scalar.activation(weff_sb[:, j], psT[:], Copy,
                                     scale=1.0 / float(np.sqrt(DQ)))
            else:
                nc.scalar.activation(weff_sb[:, j], psT[:], Copy)

    # -------- prefetch W_O + small tensors (DMA overlaps attention) ----------
    nc.sync.dma_start(wot_sb[:], io["wot"].rearrange("(g p) e -> p g e", p=P))
    for i, name in enumerate(("g1", "b1", "g2", "b2", "l2b")):
        nc.sync.dma_start(gb_sb[:, i, :], io[name].rearrange("(g p) -> p g", p=P))
    nc.sync.dma_start(l1b_sb[:], io["l1b"].rearrange("(g p) -> p g", p=P))
    nc.sync.dma_start(eres_sb[:], io["eresT"].rearrange("(g p) s -> p g s", p=P))
    nc.sync.dma_start(eresb_sb[:], io["eresb"].rearrange("(g p) s -> p g s", p=P))

    a2a_in = drampool.tile([NC * EPC, SL], bf16, tag="a2a_in", name="a2a_in")
    a2a_out = drampool.tile([NC * EPC, SL], bf16, tag="a2a_out", name="a2a_out")

    # ------- fused per-strip qkv projection + causal attention ---------------
    # Strip sp: project q/k for queries in the strip and v for its key tiles
    # (v computed directly in [keys, e] layout: eT-tile stationary, weffT
    # moving -- no transposes), then run attention for key tiles 0..4sp+3.
    # exp runs on ACT (the strip-phase bottleneck), all copies/masks on DVE,
    # o-matmuls lag score-matmuls by GD tiles so PE/ACT/DVE pipeline.
    GD = 2
    with tc.tile_pool(name="expool", bufs=6) as expool, \
         tc.tile_pool(name="scpsum", bufs=2, space="PSUM") as scpsum, \
         tc.tile_pool(name="opsum", bufs=1, space="PSUM") as opsum, \
         tc.tile_pool(name="qkpsum", bufs=1, space="PSUM") as qkpsum, \
         tc.tile_pool(name="pvpsum", bufs=1, space="PSUM") as pvpsum, \
         tc.tile_pool(name="nrmpool", bufs=2) as nrmpool:
        carry = None          # (sp, tmax, pso, exs) pending from previous strip

        def finish_strip(sp0, tmax0, pso0, exs0):
            qs0 = slice(sp0 * SPW, (sp0 + 1) * SPW)
            for t2 in range(tmax0 - GD, tmax0):
                ex2 = exs0.pop(t2)
                for h in range(HPC):
                    nc.tensor.matmul(pso0[h][:], vaug_sb[:, h, t2, :],
                                     ex2[:, h, :],
                                     start=(t2 == 0), stop=(t2 == tmax0 - 1))
            for h in range(HPC):
                hs = slice(h * DQ, (h + 1) * DQ)
                rc = nrmpool.tile([1, SPW], fp32, tag="rc")
                nc.vector.reciprocal(rc[:], pso0[h][DQ:DQ + 1, :])
                rcb = nrmpool.tile([DQ, SPW], fp32, tag="rcb")
                nc.gpsimd.partition_broadcast(rcb[:], rc[:], channels=DQ)
                nc.vector.tensor_mul(ot_sb[hs, qs0], pso0[h][:DQ, :], rcb[:])
            # stage this strip's heads-block into the AllToAll source buffer
            # (gpsimd queue: the ACT queue must keep streaming exps)
            nc.gpsimd.dma_start(
                a2a_in[HPC * sp0 * EPC:HPC * (sp0 + 1) * EPC, :].rearrange(
                    "(g e) s -> e g s", g=HPC),
                ot_sb[:, qs0].rearrange("e (g s) -> e g s", g=HPC))

        def emit_qkv(sp):
            qs = slice(sp * SPW, (sp + 1) * SPW)
            for j, dst in ((0, qt_sb), (1, kt_sb)):
                pj = qkpsum.tile([EPC, SPW], fp32, tag="pj", name="pj")
                for kk in range(DK):
                    nc.tensor.matmul(pj[:], weff_sb[:, j, kk, :],
                                     et_sb[:, kk, qs],
                                     start=(kk == 0), stop=(kk == DK - 1))
                nc.vector.tensor_copy(dst[:, qs], pj[:])
            for ti in range(TPS):
                t = TPS * sp + ti
                pv = pvpsum.tile([P, EPC], fp32, tag="pv", name="pv")
                for kk in range(DK):
                    nc.tensor.matmul(pv[:], et_sb[:, kk, ts(t, P)],
                                     weff_sb[:, 2, kk, :],
                                     start=(kk == 0), stop=(kk == DK - 1))
                nc.vector.tensor_copy(vaug_sb[:, :, t, :DQ],
                                      pv.rearrange("p (h e) -> p h e", h=HPC))

        for sp in range(NSP):
            qs = slice(sp * SPW, (sp + 1) * SPW)
            emit_qkv(sp)
            if carry is not None:
                finish_strip(*carry)
            tmax = TPS * (sp + 1)          # key tiles needed by this strip
            pso = [opsum.tile([DQ + 1, SPW], fp32, tag=f"pso{h}", name=f"pso{h}")
                   for h in range(HPC)]
            exs = {}
            for tt in range(tmax):
                if tt < tmax:
                    t = tt
                    ps = scpsum.tile([P, HPC, SPW], fp32, tag="scps")
                    for h in range(HPC):
                        hs = slice(h * DQ, (h + 1) * DQ)
                        nc.tensor.matmul(ps[:, h, :], kt_sb[hs, ts(t, P)],
                                         qt_sb[hs, qs], start=True, stop=True)
                    ex = expool.tile([P, HPC, SPW], bf16, tag="ex")
                    nc.scalar.activation(ex[:], ps[:], Exp)
                    if t >= TPS * sp:      # diagonal tile: causal mask in-place
                        for h in range(HPC):
                            nc.vector.tensor_mul(ex[:, h, :], ex[:, h, :],
                                                 tri_sb[:, t - TPS * sp, :])
                    exs[t] = ex
                if tt >= GD:
                    t2 = tt - GD
                    ex2 = exs.pop(t2)
                    for h in range(HPC):
                        nc.tensor.matmul(pso[h][:], vaug_sb[:, h, t2, :],
                                         ex2[:, h, :],
                                         start=(t2 == 0), stop=(t2 == tmax - 1))
            carry = (sp, tmax, pso, exs)
        finish_strip(*carry)
    etctx.close()

    if stop_after in ("qkv", "attn"):
        _ctx.close(); return

    # ---------------- AllToAll: heads-sharded -> token-sharded ----------------
    a2a_in = drampool.tile([NC * EPC, SL], bf16, tag="a2a_in", name="a2a_in")
    a2a_out = drampool.tile([NC * EPC, SL], bf16, tag="a2a_out", name="a2a_out")
    nc.scalar.dma_start(a2a_in.rearrange("(g e) s -> e g s", g=NC),
                        ot_sb.rearrange("e (g s) -> e g s", g=NC))
    if use_collective:
        nc.gpsimd.collective_compute(
            "AllToAll", mybir.AluOpType.bypass,
            replica_groups=[list(range(NC))],
            ins=[a2a_in.opt()], outs=[a2a_out.opt()])
    else:  # timing-only single-core variant (results wrong across cores)
        nc.scalar.dma_start(a2a_out[:], a2a_in[:])
    nc.scalar.dma_start(ots_sb[:], a2a_out.rearrange("(g e) s -> e g s", g=NC))

    # ------- W_O projection (attT), LN1 statistics interleaved per chunk -----
    ln1ctx = contextlib.ExitStack()
    ln1psum = ln1ctx.enter_context(tc.tile_pool(name="ln1psum", bufs=1, space="PSUM"))
    ln1ps1 = ln1psum.tile([1, SL], fp32, tag="ln1ps1", name="ln1ps1")
    ln1ps2 = ln1psum.tile([1, SL], fp32, tag="ln1ps2", name="ln1ps2")
    with tc.tile_pool(name="atpsum", bufs=2, space="PSUM") as atpsum, \
         tc.tile_pool(name="wosq", bufs=2) as wosq:
        for m in range(DK):
            ps = atpsum.tile([P, SL], fp32, tag="atps")
            for g in range(DK):
                nc.tensor.matmul(ps[:], wot_sb[:, g, ts(m, P)], ots_sb[:, g, :],
                                 start=(g == 0), stop=(g == DK - 1))
            nc.scalar.activation(att_sb[:, m, :], ps[:], Copy)
            sq = wosq.tile([P, SL], bf16, tag="sq")
            nc.vector.tensor_mul(sq[:], att_sb[:, m, :], att_sb[:, m, :])
            nc.tensor.matmul(ln1ps1[:], onesb_sb[:], att_sb[:, m, :],
                             start=(m == 0), stop=(m == DK - 1))
            nc.tensor.matmul(ln1ps2[:], onesb_sb[:], sq[:],
                             start=(m == 0), stop=(m == DK - 1))

    if stop_after == "wo":
        _ctx.close(); return

    # ------- LayerNorm application (stats psums precomputed upstream) --------
    def ln_apply(ps1, ps2, gi, bi, src_sb, res_sb, dsts, pools, out_cb=None,
                 res_b=None, dst_b=None):
        """dsts = res + gamma*(src-mean)/sqrt(var+eps) + beta.

        src_sb: [P, DK, SL] bf16; res_sb: [P, DK, SL] f32; ps1/ps2: [1, SL]
        sum / sum-of-squares psums; out_cb(g) runs after chunk g is written.
        """
        sqpool, stpool = pools
        mean = stpool.tile([1, SL], fp32, tag="mean")
        nc.vector.tensor_scalar_mul(mean[:], ps1[:], 1.0 / D)
        var = stpool.tile([1, SL], fp32, tag="var")
        nc.vector.tensor_scalar_mul(var[:], ps2[:], 1.0 / D)
        msq = stpool.tile([1, SL], fp32, tag="msq")
        nc.vector.tensor_mul(msq[:], mean[:], mean[:])
        nc.vector.tensor_tensor(var[:], var[:], msq[:], sub_op)
        nc.vector.tensor_scalar_add(var[:], var[:], EPS)
        std = stpool.tile([1, SL], fp32, tag="std")
        nc.scalar.activation(std[:], var[:], Sqrt)
        rstd = stpool.tile([1, SL], fp32, tag="rstd")
        nc.vector.reciprocal(rstd[:], std[:])
        mrs = stpool.tile([1, SL], fp32, tag="mrs")  # mean*rstd
        nc.vector.tensor_mul(mrs[:], mean[:], rstd[:])
        rstdh = stpool.tile([1, SL], bf16, tag="rstdh")
        nc.vector.tensor_copy(rstdh[:], rstd[:])
        mrsh = stpool.tile([1, SL], bf16, tag="mrsh")
        nc.vector.tensor_copy(mrsh[:], mrs[:])
        rstdb = stpool.tile([P, SL], bf16, tag="rstdb")
        nc.gpsimd.partition_broadcast(rstdb[:], rstdh[:])
        mrsb = stpool.tile([P, SL], bf16, tag="mrsb")
        nc.gpsimd.partition_broadcast(mrsb[:], mrsh[:])
        t1s = []
        for g in range(DK):
            t1 = sqpool.tile([P, SL], bf16, tag="t1", name="t1")
            nc.vector.tensor_mul(t1[:], src_sb[:, g, :], rstdb[:])
            nc.vector.tensor_tensor(t1[:], t1[:], mrsb[:], sub_op)
            nc.vector.tensor_scalar(t1[:], t1[:], gb_sb[:, gi, g:g + 1],
                                    gb_sb[:, bi, g:g + 1], mult_op, add_op)
            if dst_b is not None:   # fast bf16 residual add -> consumers start
                nc.vector.tensor_tensor(dst_b[:, g, :], t1[:], res_b[:, g, :],
                                        add_op)
                t1s.append(t1)
            else:
                dst0 = dsts[0]
                nc.vector.tensor_tensor(dst0[:, g, :], t1[:], res_sb[:, g, :],
                                        add_op)
                for dst in dsts[1:]:
                    nc.scalar.activation(dst[:, g, :], dst0[:, g, :], Copy)
            if out_cb is not None:
                out_cb(g)
        if dst_b is not None:       # deferred f32 path (needed only at LN2)
            for g in range(DK):
                nc.vector.tensor_tensor(dsts[0][:, g, :], t1s[g][:],
                                        res_sb[:, g, :], add_op)

    with tc.tile_pool(name="sqpool", bufs=9) as sqpool, \
         tc.tile_pool(name="stpool", bufs=2) as stpool:
        ln_apply(ln1ps1, ln1ps2, 0, 1, att_sb, eres_sb,
                 [x1t_sb], (sqpool, stpool), res_b=eresb_sb, dst_b=x1b_sb)
    ln1ctx.close()

    if stop_after == "ln1":
        _ctx.close(); return

    # ---------------- MLP (L1/L2 streamed; LN2 stats inside MLP2 loop) -------
    MCG = 4                       # mc columns per streamed L1 chunk
    ln2psum = _ctx.enter_context(tc.tile_pool(name="ln2psum", bufs=1, space="PSUM"))
    ln2ps1 = ln2psum.tile([1, SL], fp32, tag="ln2ps1", name="ln2ps1")
    ln2ps2 = ln2psum.tile([1, SL], fp32, tag="ln2ps2", name="ln2ps2")
    with tc.tile_pool(name="hallpool", bufs=1) as hallpool, \
         tc.tile_pool(name="mlsq", bufs=2) as mlsq, \
         tc.tile_pool(name="hpsum", bufs=4, space="PSUM") as hpsum:
        ht_all = hallpool.tile([P, MK, SL], bf16, tag="ht_all")
        for mcg in range(MK // MCG):
            l1c = l1pool.tile([P, DK, MCG * P], bf16, tag="l1c")
            nc.sync.dma_start(
                l1c[:], io["l1t"][:, ts(mcg, MCG * P)].rearrange(
                    "(g p) m -> p g m", p=P))
            for mi in range(MCG):
                mc = mcg * MCG + mi
                psh = hpsum.tile([P, SL], fp32, tag="psh")
                for g in range(DK):
                    nc.tensor.matmul(psh[:], l1c[:, g, ts(mi, P)], x1b_sb[:, g, :],
                                     start=(g == 0), stop=(g == DK - 1))
                nc.scalar.activation(ht_all[:, mc, :], psh[:], Relu,
                                     bias=l1b_sb[:, mc:mc + 1])
        with tc.tile_pool(name="mlppsum", bufs=2, space="PSUM") as mlppsum:
            for dt in range(DK):
                l2td = l2pool.tile([P, MK, P], bf16, tag="l2td")
                nc.sync.dma_start(l2td[:], io["l2t"][dt])
                psm2 = mlppsum.tile([P, SL], fp32, tag="psm2")
                for mc in range(MK):
                    nc.tensor.matmul(psm2[:], l2td[:, mc, :], ht_all[:, mc, :],
                                     start=(mc == 0), stop=(mc == MK - 1))
                nc.vector.tensor_scalar_add(mlp_sb[:, dt, :], psm2[:],
                                            gb_sb[:, 4, dt:dt + 1])
                sq = mlsq.tile([P, SL], bf16, tag="sq")
                nc.vector.tensor_mul(sq[:], mlp_sb[:, dt, :], mlp_sb[:, dt, :])
                nc.tensor.matmul(ln2ps1[:], onesb_sb[:], mlp_sb[:, dt, :],
                                 start=(dt == 0), stop=(dt == DK - 1))
                nc.tensor.matmul(ln2ps2[:], onesb_sb[:], sq[:],
                                 start=(dt == 0), stop=(dt == DK - 1))

    if stop_after == "mlp":
        _ctx.close(); return

    # ---------------- LN2 + residual; output stays [d, s] (host re-lays out) ----
    with tc.tile_pool(name="sqpool2", bufs=3) as sqpool2, \
         tc.tile_pool(name="stpool2", bufs=2) as stpool2, \
         tc.tile_pool(name="outpool", bufs=1) as outpool:
        outT = outpool.tile([P, DK, SL], fp32, tag="outT")
        outdram = io["out"].rearrange("(g p) s -> p g s", p=P)

        def flush(g):
            if g % 2 == 1:     # stream result out two chunks at a time
                nc.scalar.dma_start(outdram[:, g - 1:g + 1, :],
                                    outT[:, g - 1:g + 1, :])

        ln_apply(ln2ps1, ln2ps2, 2, 3, mlp_sb, x1t_sb, [outT],
                 (sqpool2, stpool2), out_cb=flush)
    _ctx.close()


def build_program(S=S_FULL, use_collective=True, stop_after=None):
    import concourse.mybir as mybir
    import concourse.tile as tile
    from concourse import bacc

    nc = bacc.Bacc("TRN2", target_bir_lowering=False, debug=False,
                   enable_asserts=True, num_devices=NC if use_collective else 1)
    f32, bf16 = mybir.dt.float32, mybir.dt.bfloat16

    def din(name, shape, dt=bf16):
        return nc.dram_tensor(name, shape, dt, kind="ExternalInput").ap()

    io = {
        "eT": din("eT", [D, S]),
        "eresT": din("eresT", [D, S // NC], f32),
        "eresb": din("eresb", [D, S // NC]),
        "wq": din("wq", [D, D]), "wk": din("wk", [D, D]),
        "wv": din("wv", [D, D]),
        "htq": din("htq", [D, EPC]), "htk": din("htk", [D, EPC]),
        "htv": din("htv", [D, EPC]),
        "wot": din("wot", [D, D]),
        "l1t": din("l1t", [D, DM]), "l2t": din("l2t", [DK, P, MK, P]),
        "l1b": din("l1b", [DM], f32), "l2b": din("l2b", [D], f32),
        "g1": din("g1", [D], f32), "b1": din("b1", [D], f32),
        "g2": din("g2", [D], f32), "b2": din("b2", [D], f32),
        "out": nc.dram_tensor("out", [D, S // NC], f32, kind="ExternalOutput").ap(),
    }
    with tile.TileContext(nc) as tc:
        _body(tc, io, S, use_collective, stop_after)
    nc.compile()
    return nc


def make_in_maps(E, mask, W_Q, W_K, W_V, W_O, H_Q, H_K, H_V,
                 L1_w, L1_b, L2_w, L2_b, gamma1, beta1, gamma2, beta2, S=S_FULL):
    E = np.asarray(E, np.float32).reshape(S, D)
    SL = S // NC
    com = {
        "eT": np.ascontiguousarray(E.T).astype(BF16),
        "wq": np.asarray(W_Q, np.float32).astype(BF16),
        "wk": np.asarray(W_K, np.float32).astype(BF16),
        "wv": np.asarray(W_V, np.float32).astype(BF16),
        "wot": np.ascontiguousarray(np.asarray(W_O, np.float32).T).astype(BF16),
        "l1t": np.ascontiguousarray(np.asarray(L1_w, np.float32).T).astype(BF16),
        "l2t": np.ascontiguousarray(
            np.asarray(L2_w, np.float32).T.reshape(MK, P, DK, P).transpose(2, 1, 0, 3)
        ).astype(BF16),
        "l1b": np.asarray(L1_b, np.float32), "l2b": np.asarray(L2_b, np.float32),
        "g1": np.asarray(gamma1, np.float32), "b1": np.asarray(beta1, np.float32),
        "g2": np.asarray(gamma2, np.float32), "b2": np.asarray(beta2, np.float32),
    }
    in_maps = []
    for c in range(NC):
        m = dict(com)
        m["eresT"] = np.ascontiguousarray(E[c * SL:(c + 1) * SL, :].T)
        m["eresb"] = m["eresT"].astype(BF16)
        for key, Hw in (("htq", H_Q), ("htk", H_K), ("htv", H_V)):
            hs = np.asarray(Hw, np.float32)[c * HPC:(c + 1) * HPC].reshape(EPC, D)
            m[key] = np.ascontiguousarray(hs.T).astype(BF16)
        in_maps.append(m)
    return in_maps


_PROGRAM_CACHE = {}


def kernel(**inputs):
    from concourse import bass_utils
    S = inputs["E"].shape[1]
    if S not in _PROGRAM_CACHE:
        _PROGRAM_CACHE[S] = build_program(S)
    nc = _PROGRAM_CACHE[S]
    in_maps = make_in_maps(S=S, **inputs)
    res = bass_utils.run_bass_kernel_spmd(nc, in_maps, core_ids=list(range(NC)))
    SL = S // NC
    out = np.concatenate([np.ascontiguousarray(res.results[c]["out"].T)
                          for c in range(NC)], axis=0)
    return out.reshape(1, S, D).astype(np.float32)
